# revision 41
# baseline (speedup 1.0000x reference)
"""Two-layer GCN encoder (PyG GCNConv x2 + sinusoidal PE + ReLU) on 8 TRN2
NeuronCores via Bass/Tile.

Strategy (node sharding, per spec sharding_hint):
  - 50000 nodes are packed into 8 cores x 49 tiles x 128 slots, balanced by
    in-degree so every 128-target tile has ~equal edge count; within a core,
    tiles are ordered by descending edge count so the per-tile-rank static
    gather sizes (shared by all cores, SPMD) carry minimal padding.
  - GCN algebra: out = dinv * (A+I)_hat @ (dinv * h) with h = x @ W. For
    layer 1 the aggregation commutes with the GEMM, so we aggregate the
    64-dim scaled inputs first and apply W1 after (4x less gather traffic).
  - Per output tile: dma_gather pulls the source rows for its edges
    (calls of <=1024 indices, split across the two half-tables since gather
    indices are int16), a one-hot selection matrix S[e,t]=(tgt_local[e]==t)
    built with DVE is_equal against an iota row turns the segment-sum into
    PE matmuls accumulated in PSUM (transposed: agg^T = G^T @ S, so features
    land on partitions and the layer GEMMs need no transposes).
  - Self-loops never touch the gather path: one PE matmul against the
    identity adds the tile's own rows into the aggregation.
  - The shared gather table (scaled activations of ALL nodes) is exchanged
    between layers with two 8-core AllGathers over the node halves of the
    shard, so the first half-table's layer-2 gathers overlap the second
    AllGather.

kernel(**inputs) takes the FULL unsharded inputs and returns the FULL
[50000, 256] float32 output; all sharding happens inside.
"""
import heapq
from contextlib import ExitStack

import numpy as np

import concourse.bacc as bacc
import concourse.mybir as mybir
import concourse.tile as tile
from concourse import bass_utils
from concourse.masks import make_identity

N_CORES = 8
P = 128
IN_DIM = 64
HID = 256
SUB_MAX = 8  # max chunks (of 128 idxs) per dma_gather call: 1024-idx HW cap

_PROGRAM_CACHE = {}
_last_results = None  # BassKernelResults of the most recent run (for test.py)


# ----------------------------------------------------------------------------
# host-side helpers
# ----------------------------------------------------------------------------

def _sinusoidal_pe(n, d):
    pos = np.arange(n, dtype=np.float32)[:, None]
    div = np.exp(
        np.arange(0, d, 2, dtype=np.float32) * np.float32(-np.log(10000.0) / d)
    ).astype(np.float32)
    ang = pos * div[None, :]
    pe = np.zeros((n, d), dtype=np.float32)
    pe[:, 0::2] = np.sin(ang)
    pe[:, 1::2] = np.cos(ang)
    return pe


def _pack_bins(w, n_bins):
    """Greedy LPT with capacity: each node to a (bin, slot), <=128 nodes per
    bin, balancing per-bin weight sums."""
    n = w.shape[0]
    bin_of = np.empty(n, dtype=np.int64)
    slot_of = np.empty(n, dtype=np.int64)
    counts = np.zeros(n_bins, dtype=np.int64)
    heap = [(0, b) for b in range(n_bins)]
    heapq.heapify(heap)
    order = np.argsort(-w, kind="stable")
    for i in order:
        while True:
            s, b = heapq.heappop(heap)
            if counts[b] < P:
                break
        bin_of[i] = b
        slot_of[i] = counts[b]
        counts[b] += 1
        if counts[b] < P:
            heapq.heappush(heap, (s + int(w[i]), b))
    return bin_of, slot_of


def _wrap_idxs(flat):
    """[m*16] int -> [128, m] int16 wrapped: idx i at partition i%16
    (replicated across the 8 groups of 16 partitions), column i//16."""
    a = np.asarray(flat, dtype=np.int16).reshape(-1, 16).T
    return np.tile(a, (8, 1))


def _ag_splits(TPC, HT):
    """Incremental-AllGather piece list [(half, tile_lo, tile_hi)].
    The table layout is piece-major (each AG piece's 8-core rows are a
    contiguous block), so kernel and host must use the same splits."""
    def spl(n, k):
        base, rem = divmod(n, k)
        out, s = [], 0
        for i in range(k):
            e = s + base + (1 if i < rem else 0)
            out.append((s, e))
            s = e
        return out
    return ([(0, s, e) for s, e in spl(HT, 1)]
            + [(1, s, e) for s, e in spl(TPC - HT, 1)])


# ----------------------------------------------------------------------------
# device program
# ----------------------------------------------------------------------------

def _build_program(geom):
    """One SPMD program for all 8 cores.

    geom: (TPC, tuple over tile-rank of (szA, szB)) where szH = static idx
    count (multiple of 128) of the gather against table half H for the
    tile at that rank."""
    TPC, HT0, sizes = geom
    S_PAD = TPC * P
    S_A = HT0 * P                  # per-core rows in node-half 0
    S_B = S_PAD - S_A
    f32 = mybir.dt.float32

    # per-tile chunk counts and buffer offsets
    cpt = [(a + b) // P for a, b in sizes]           # chunks per tile
    tloc_off = np.concatenate([[0], np.cumsum(cpt)]).astype(int)
    TCOLS = int(tloc_off[-1])
    icols_t = [c * 8 for c in cpt]                   # idx cols per tile
    idx_off = np.concatenate([[0], np.cumsum(icols_t)]).astype(int)
    ICOLS = int(idx_off[-1])

    nc = bacc.Bacc("TRN2", target_bir_lowering=False, debug=False,
                   num_devices=N_CORES, num_swdge_queues=4)
    qctr = [0]

    def next_q():
        q = qctr[0] % 4
        qctr[0] += 1
        return q

    bf16 = mybir.dt.bfloat16
    TOT = N_CORES * S_PAD
    # replicated, host-prepared gather table for layer 1: row tpos[node] =
    # dinv[node] * x[node] in cols [0, IN_DIM), zeros beyond (pad to 128
    # cols so the 256B gather-elem constraint holds for bf16)
    t1t_d = nc.dram_tensor("t1t", [TOT, P], bf16, kind="ExternalInput")
    # per-core slice of t1t (this core's own nodes) for the self-loop path
    self_d = nc.dram_tensor("selfr", [S_PAD, IN_DIM], bf16,
                            kind="ExternalInput")
    dinvB_d = nc.dram_tensor("dinvB", [P, S_PAD], f32, kind="ExternalInput")
    peT_d = nc.dram_tensor("peT", [HID, S_PAD], f32, kind="ExternalInput")
    idx_d = nc.dram_tensor("idxb", [P, ICOLS], mybir.dt.int16,
                           kind="ExternalInput")
    tloc_d = nc.dram_tensor("tlocb", [P, TCOLS], f32, kind="ExternalInput")
    w1_d = nc.dram_tensor("W1", [IN_DIM, HID], bf16, kind="ExternalInput")
    w2_d = nc.dram_tensor("W2", [HID, HID], f32, kind="ExternalInput")
    b1_d = nc.dram_tensor("b1", [HID], f32, kind="ExternalInput")
    b2_d = nc.dram_tensor("b2", [HID], f32, kind="ExternalInput")
    outT_d = nc.dram_tensor("outT", [HID, S_PAD], f32, kind="ExternalOutput")

    rg = [list(range(N_CORES))]

    def sub_splits(sz):
        """split an idx count (multiple of 128) into chunks-groups <= SUB_MAX"""
        ch = sz // P
        subs = []
        off = 0
        while ch > 0:
            take = min(ch, SUB_MAX)
            subs.append((off, take))
            off += take
            ch -= take
        return subs

    with tile.TileContext(nc) as tc, ExitStack() as ctx:
        dram = ctx.enter_context(tc.tile_pool(name="dram", bufs=1, space="DRAM"))
        # layer-2 shard buffers split by (asymmetric) node-half so the
        # mid-layer AllGather of half 0 has no dependency on later tiles
        t2_in = [dram.tile([S_A, HID], bf16, name="t2in0"),
                 dram.tile([S_B, HID], bf16, name="t2in1")]
        A_TOT = N_CORES * S_A
        BB = max(TOT - 32768, 0)  # base row of gather piece B (idx < 32768)
        t2t = dram.tile([TOT, HID], bf16, name="t2t")
        # piece A reads only AG-a rows (so it never waits for the second AG)
        t1h = [t1t_d[0:A_TOT, :], t1t_d[BB:, :]]
        t2h = [t2t[0:A_TOT, :], t2t[BB:, :]]
        HT = HT0  # tiles in node-half 0

        def half_slice(j):
            """(half tensor index, row slice within it) for tile j"""
            h = 0 if j < HT else 1
            base = (j - h * HT) * P
            return h, slice(base, base + P)

        const = ctx.enter_context(tc.tile_pool(name="const", bufs=1))

        iota = const.tile([P, P], f32)
        nc.gpsimd.iota(iota[:], pattern=[[1, P]], base=0,
                       channel_multiplier=0,
                       allow_small_or_imprecise_dtypes=True)
        ident = const.tile([P, P], f32)
        make_identity(nc, ident[:])
        ident_bf = const.tile([P, P], mybir.dt.bfloat16)
        nc.vector.tensor_copy(out=ident_bf[:], in_=ident[:])
        w1 = const.tile([IN_DIM, HID], bf16)
        nc.sync.dma_start(out=w1[:], in_=w1_d[:])
        w2k = []
        for k in range(2):
            t = const.tile([P, HID], f32, name=f"w2k{k}")
            nc.sync.dma_start(out=t[:], in_=w2_d[k * P:(k + 1) * P, :])
            w2k.append(t)
        b1t, b2t = [], []
        for m in range(2):
            t = const.tile([P, 1], f32, name=f"b1t{m}")
            nc.sync.dma_start(out=t[:], in_=b1_d[m * P:(m + 1) * P, None])
            b1t.append(t)
            t = const.tile([P, 1], f32, name=f"b2t{m}")
            nc.sync.dma_start(out=t[:], in_=b2_d[m * P:(m + 1) * P, None])
            b2t.append(t)
        idx_sb = const.tile([P, ICOLS], mybir.dt.int16)
        nc.sync.dma_start(out=idx_sb[:], in_=idx_d[:])
        tloc_sb = const.tile([P, TCOLS], f32)
        nc.sync.dma_start(out=tloc_sb[:], in_=tloc_d[:])
        dinvB_sb = const.tile([P, S_PAD], f32)
        nc.sync.dma_start(out=dinvB_sb[:], in_=dinvB_d[:])

        # ---- layer 1 -> t2_in; incremental AllGather pieces fire as tile
        # groups complete (piece-major table layout: contiguous out slices)
        ag_pieces = _ag_splits(TPC, HT)
        ag_bases = []
        pb = 0
        for _, s, e in ag_pieces:
            ag_bases.append(pb)
            pb += N_CORES * (e - s) * P

        with tc.tile_pool(name="g1", bufs=6) as g1p, \
             tc.tile_pool(name="s1", bufs=4) as s1p, \
             tc.tile_pool(name="wk1", bufs=3) as wk, \
             tc.tile_pool(name="agg1", bufs=2, space="PSUM") as agg1p, \
             tc.tile_pool(name="mm1", bufs=2, space="PSUM") as mm1p, \
             tc.tile_pool(name="trp", bufs=2, space="PSUM") as trpp:
            for j in range(TPC):
                jr = slice(j * P, (j + 1) * P)
                nchunk = cpt[j]
                gb = g1p.tile([P, nchunk, P], bf16, tag="gb",
                              name=f"gb_{j}")
                for h in range(2):
                    base = 0 if h == 0 else sizes[j][0] // P
                    c0 = int(idx_off[j]) + base * 8
                    for off, take in sub_splits(sizes[j][h]):
                        nc.gpsimd.dma_gather(
                            out_ap=gb[:, base + off:base + off + take, :],
                            in_ap=t1h[h],
                            idxs_ap=idx_sb[:, c0 + off * 8:c0 + (off + take) * 8],
                            num_idxs=take * P, num_idxs_reg=take * P,
                            elem_size=P, queue_num=next_q())
                # self rows (this tile's own nodes), avoids self-loop gathers
                hh, hr = half_slice(j)
                selfr = wk.tile([P, IN_DIM], bf16, tag="selfr")
                nc.sync.dma_start(out=selfr[:], in_=self_d[jr, :])
                aggp = agg1p.tile([IN_DIM, P], f32, tag="agg")
                nc.tensor.matmul(out=aggp[:], lhsT=selfr[:], rhs=ident_bf[:],
                                 start=True, stop=False)
                # one batched is_equal builds the whole tile's S matrices
                # (amortizes the ~150ns DVE dispatch over nchunk slices)
                col = int(tloc_off[j])
                S3 = s1p.tile([P, nchunk, P], bf16, tag="S",
                              name=f"S3_{j}")
                nc.vector.tensor_tensor(
                    out=S3[:],
                    in0=tloc_sb[:, col:col + nchunk, None].to_broadcast(
                        [P, nchunk, P]),
                    in1=iota[:, None, :].to_broadcast([P, nchunk, P]),
                    op=mybir.AluOpType.is_equal)
                for g in range(nchunk):
                    nc.tensor.matmul(out=aggp[:], lhsT=gb[:, g, :IN_DIM],
                                     rhs=S3[:, g, :],
                                     start=False, stop=(g == nchunk - 1))
                aggs = wk.tile([IN_DIM, P], bf16, tag="aggs")
                nc.vector.tensor_tensor(out=aggs[:], in0=aggp[:],
                                        in1=dinvB_sb[:IN_DIM, jr],
                                        op=mybir.AluOpType.mult)
                xw = wk.tile([P, HID], mybir.dt.bfloat16, tag="xw")
                for m in range(2):
                    mr = slice(m * P, (m + 1) * P)
                    o1 = mm1p.tile([P, P], f32, tag="o1")
                    nc.tensor.matmul(out=o1[:], lhsT=w1[:, mr], rhs=aggs[:],
                                     start=True, stop=True)
                    pet = wk.tile([P, P], f32, tag="pet")
                    nc.sync.dma_start(out=pet[:], in_=peT_d[mr, jr])
                    tsb = wk.tile([P, P], f32, tag="tsb")
                    nc.vector.tensor_tensor(out=tsb[:], in0=o1[:], in1=pet[:],
                                            op=mybir.AluOpType.add)
                    x2s = wk.tile([P, P], f32, tag="x2s")
                    nc.scalar.activation(
                        out=x2s[:], in_=tsb[:],
                        func=mybir.ActivationFunctionType.Relu,
                        bias=b1t[m][:, :1], scale=1.0)
                    nc.vector.tensor_tensor(out=x2s[:], in0=x2s[:],
                                            in1=dinvB_sb[:, jr],
                                            op=mybir.AluOpType.mult)
                    trt = trpp.tile([P, P], f32, tag="tr")
                    nc.tensor.transpose(out=trt[:], in_=x2s[:],
                                        identity=ident[:])
                    nc.vector.tensor_copy(out=xw[:, mr], in_=trt[:])
                nc.sync.dma_start(out=t2_in[hh][hr, :], in_=xw[:])
                # incremental AllGather: ship each completed tile-group of
                # the shard as soon as its t2_in rows are written, so the
                # full table is ready right as layer 1 drains
                for pi, (ph, ps, pe_) in enumerate(ag_pieces):
                    if (j + 1 == (ph * HT) + pe_) and (hh == ph):
                        pr = slice(ps * P, pe_ * P)
                        rows = N_CORES * (pe_ - ps) * P
                        nc.gpsimd.collective_compute(
                            "AllGather", mybir.AluOpType.bypass,
                            replica_groups=rg,
                            ins=[t2_in[ph][pr, :]],
                            outs=[t2t[ag_bases[pi]:ag_bases[pi] + rows, :]])

        # ---- layer 2 -> outT
        # piece-A gathers run W tiles ahead of the per-tile B-gather +
        # compute pass, so the stream keeps the SWDGE queues fed while
        # waiting for the half-1 AllGather to land
        W2PF = 10
        with tc.tile_pool(name="g2", bufs=W2PF + 2) as g2p, \
             tc.tile_pool(name="s2", bufs=4) as s2p, \
             tc.tile_pool(name="wk2", bufs=3) as wk2, \
             tc.tile_pool(name="agg2", bufs=2, space="PSUM") as agg2p, \
             tc.tile_pool(name="mm2", bufs=4, space="PSUM") as mm2p:
            gb2s = {}

            def l2_gathers(j, h):
                gb2 = gb2s[j]
                base = 0 if h == 0 else sizes[j][0] // P
                c0 = int(idx_off[j]) + base * 8
                for off, take in sub_splits(sizes[j][h]):
                    nc.gpsimd.dma_gather(
                        out_ap=gb2[:, base + off:base + off + take, :],
                        in_ap=t2h[h],
                        idxs_ap=idx_sb[:, c0 + off * 8:c0 + (off + take) * 8],
                        num_idxs=take * P, num_idxs_reg=take * P,
                        elem_size=HID, queue_num=next_q())

            def l2_tile(j):
                jr = slice(j * P, (j + 1) * P)
                nchunk = cpt[j]
                gb2 = gb2s.pop(j)
                hh, hr = half_slice(j)
                self2 = wk2.tile([P, HID], mybir.dt.bfloat16, tag="self2")
                nc.sync.dma_start(out=self2[:], in_=t2_in[hh][hr, :])
                a2 = [agg2p.tile([P, P], f32, tag=f"a2_{k}", name=f"a2_{k}_{j}")
                      for k in range(2)]
                for k in range(2):
                    nc.tensor.matmul(out=a2[k][:],
                                     lhsT=self2[:, k * P:(k + 1) * P],
                                     rhs=ident_bf[:], start=True, stop=False,
                                     skip_group_check=True)
                col = int(tloc_off[j])
                S3 = s2p.tile([P, nchunk, P], mybir.dt.bfloat16, tag="S2",
                              name=f"S3b_{j}")
                nc.vector.tensor_tensor(
                    out=S3[:],
                    in0=tloc_sb[:, col:col + nchunk, None].to_broadcast(
                        [P, nchunk, P]),
                    in1=iota[:, None, :].to_broadcast([P, nchunk, P]),
                    op=mybir.AluOpType.is_equal)
                for g in range(nchunk):
                    for k in range(2):
                        nc.tensor.matmul(
                            out=a2[k][:], lhsT=gb2[:, g, k * P:(k + 1) * P],
                            rhs=S3[:, g, :], start=False,
                            stop=(g == nchunk - 1),
                            skip_group_check=True)
                a2s = []
                for k in range(2):
                    t = wk2.tile([P, P], f32, tag=f"a2s{k}")
                    nc.vector.tensor_tensor(out=t[:], in0=a2[k][:],
                                            in1=dinvB_sb[:, jr],
                                            op=mybir.AluOpType.mult)
                    a2s.append(t)
                for m in range(2):
                    mr = slice(m * P, (m + 1) * P)
                    o2 = mm2p.tile([P, P], f32, tag="o2")
                    for k in range(2):
                        nc.tensor.matmul(out=o2[:], lhsT=w2k[k][:, mr],
                                         rhs=a2s[k][:], start=(k == 0),
                                         stop=(k == 1))
                    o2sb = wk2.tile([P, P], f32, tag="o2sb")
                    nc.vector.tensor_scalar_add(o2sb[:], o2[:], b2t[m][:, :1])
                    nc.sync.dma_start(out=outT_d[mr, jr], in_=o2sb[:])

            for j in range(TPC):
                gb2s[j] = g2p.tile([P, cpt[j], HID], mybir.dt.bfloat16,
                                   tag="gb2", name=f"gb2_{j}")
                l2_gathers(j, 0)
                if j >= W2PF:
                    l2_gathers(j - W2PF, 1)
                    l2_tile(j - W2PF)
            for j in range(max(TPC - W2PF, 0), TPC):
                l2_gathers(j, 1)
                l2_tile(j)

    nc.compile()
    return nc


# ----------------------------------------------------------------------------
# entry point
# ----------------------------------------------------------------------------

def _prepare(basic_block, edge_index, W1, b1, W2, b2):
    basic_block = np.ascontiguousarray(np.asarray(basic_block, dtype=np.float32))
    edge_index = np.asarray(edge_index)
    W1 = np.ascontiguousarray(np.asarray(W1, dtype=np.float32))
    W2 = np.ascontiguousarray(np.asarray(W2, dtype=np.float32))
    b1 = np.ascontiguousarray(np.asarray(b1, dtype=np.float32))
    b2 = np.ascontiguousarray(np.asarray(b2, dtype=np.float32))

    n = basic_block.shape[0]
    TPC = int(np.ceil(np.ceil(n / N_CORES) / P))
    if TPC < 2:
        TPC = 2
    S_PAD = TPC * P
    TOTp = N_CORES * S_PAD
    # asymmetric node-half split: half 0 slightly larger so the forced
    # piece-B population stays under 1024 per (tile, call); capped so the
    # piece-A row range fits int16 gather indices
    if TOTp <= 32768:
        HT0 = (TPC + 1) // 2
    else:
        HT0 = min(32768 // (N_CORES * P), TPC - 1)
    S_A = HT0 * P
    S_B = S_PAD - S_A
    NBINS = N_CORES * TPC

    src = edge_index[0].astype(np.int64)
    tgt = edge_index[1].astype(np.int64)
    e_real = src.shape[0]

    deg = np.bincount(tgt, minlength=n) + 1          # +1: self loop
    dinv = (1.0 / np.sqrt(deg)).astype(np.float32)

    # --- balanced packing (weight = gathered edges = in-degree w/o loop)
    bin_of, slot_of = _pack_bins(deg - 1, NBINS)

    # order bins within each core by ASCENDING edge count: rank sizes (max
    # over cores) stay tight, the early small tiles let the half-0
    # AllGather fire early, and the layer-1 tail is a cheap tile
    bw = np.bincount(bin_of[tgt], minlength=NBINS)   # per-bin edge count
    order = np.lexsort((bw, np.arange(NBINS) // TPC))
    # order lists bins sorted by (core, -edges); rank within core:
    rank_of_bin = np.empty(NBINS, dtype=np.int64)
    rank_of_bin[order] = np.arange(NBINS) % TPC
    tile_of = rank_of_bin[bin_of]                    # per node
    core_of = bin_of // TPC
    local_pos = tile_of * P + slot_of
    # table position: piece-major layout matching the incremental
    # AllGather pieces (each piece = contiguous 8-core block)
    A_TOT = N_CORES * S_A
    BB = max(TOTp - 32768, 0)                        # piece-B base row
    pieces = _ag_splits(TPC, HT0)
    tile_piece = np.empty(TPC, dtype=np.int64)
    piece_base = np.empty(len(pieces), dtype=np.int64)
    piece_start = np.empty(len(pieces), dtype=np.int64)
    piece_rows = np.empty(len(pieces), dtype=np.int64)
    pb = 0
    for pi, (h, s, e) in enumerate(pieces):
        g0 = h * HT0 + s
        tile_piece[g0:g0 + (e - s)] = pi
        piece_base[pi] = pb
        piece_start[pi] = g0
        piece_rows[pi] = (e - s) * P
        pb += N_CORES * (e - s) * P
    t_of = local_pos // P
    pi_of = tile_piece[t_of]
    tpos = (piece_base[pi_of] + core_of * piece_rows[pi_of]
            + (local_pos - piece_start[pi_of] * P))

    # --- per-edge call assignment: piece A covers rows [0, 32768), piece B
    # rows [BB, TOT). Sources in the overlap [BB, HALF_TOT) are flexible and
    # get assigned to whichever call balances the pair within each bin.
    eb = (core_of * TPC + tile_of)[tgt]              # bin id in rank order
    ep = tpos[src]
    fixed1 = ep >= A_TOT
    flexm = (ep >= BB) & ~fixed1
    f0 = np.bincount(eb[ep < BB], minlength=NBINS)
    f1 = np.bincount(eb[fixed1], minlength=NBINS)
    fl = np.bincount(eb[flexm], minlength=NBINS)
    x = np.clip((f1 + fl - f0 + 1) // 2, 0, fl)      # flex edges -> call A
    fidx = np.nonzero(flexm)[0]
    fsorted = fidx[np.argsort(eb[fidx], kind="stable")]
    fstart = np.concatenate([[0], np.cumsum(fl)[:-1]])
    frank = np.arange(fsorted.size) - fstart[eb[fsorted]]
    eh = np.zeros(e_real, dtype=np.int64)
    eh[fixed1] = 1
    eh[fsorted] = (frank >= x[eb[fsorted]]).astype(np.int64)
    es = ep - BB * eh                                # idx within the call
    seg = eb * 2 + eh
    osort = np.lexsort((es, seg))
    seg_s = seg[osort]
    es_s = es[osort]
    eslot_s = slot_of[tgt][osort].astype(np.float32)

    counts = np.bincount(seg, minlength=NBINS * 2).reshape(NBINS, 2)
    # static size per (tile-rank, half): max over cores, 128-aligned
    cr = counts.reshape(N_CORES, TPC, 2)
    sz = cr.max(axis=0)                              # [TPC, 2]
    sz = ((sz + P - 1) // P * P).astype(np.int64)
    sz = np.maximum(sz, P)                           # at least one chunk
    sizes = tuple((int(a), int(b)) for a, b in sz)

    # destination slot for each edge inside the padded per-(bin,half) calls
    seg_sizes = np.broadcast_to(sz[None, :, :], (N_CORES, TPC, 2)).reshape(-1)
    seg_starts = np.concatenate([[0], np.cumsum(seg_sizes)[:-1]])
    cstart = np.concatenate([[0], np.cumsum(counts.reshape(-1))[:-1]])
    rank_in_seg = np.arange(e_real) - cstart[seg_s]
    dest = seg_starts[seg_s] + rank_in_seg

    tot_slots = int(seg_sizes.sum())
    idx_pad = np.zeros(tot_slots, dtype=np.int64)    # pad -> row 0 (valid)
    idx_pad[dest] = es_s
    tloc_pad = np.full(tot_slots, -1.0, dtype=np.float32)
    tloc_pad[dest] = eslot_s

    # --- per-core input assembly
    import ml_dtypes
    bf = ml_dtypes.bfloat16
    pe = _sinusoidal_pe(n, HID)
    TOT = N_CORES * S_PAD
    gpos = core_of * S_PAD + local_pos               # global diag position
    dinv_pos = np.zeros(TOT, dtype=np.float32)
    dinv_pos[gpos] = dinv
    peT_pos = np.zeros((HID, TOT), dtype=np.float32)
    peT_pos[:, gpos] = pe.T
    # replicated layer-1 gather table (rows in AllGather piece layout) and
    # the per-core self rows, both host-scaled by dinv and cast to bf16
    xs = (basic_block * dinv[:, None]).astype(bf)
    t1table = np.zeros((TOT, P), dtype=bf)
    t1table[tpos, :IN_DIM] = xs
    selfs = np.zeros((N_CORES, S_PAD, IN_DIM), dtype=bf)
    selfs[core_of, local_pos] = xs
    W1b = W1.astype(bf)

    slots_per_core = tot_slots // N_CORES
    in_maps = []
    for c in range(N_CORES):
        cs = slice(c * S_PAD, (c + 1) * S_PAD)
        csl = slice(c * slots_per_core, (c + 1) * slots_per_core)
        idx_buf = _wrap_idxs(idx_pad[csl])
        # tloc layout: slot i of the core -> partition i%128, col i//128
        tloc_buf = np.ascontiguousarray(
            tloc_pad[csl].reshape(-1, P).T)
        in_maps.append({
            "t1t": t1table,
            "selfr": np.ascontiguousarray(selfs[c]),
            "dinvB": np.ascontiguousarray(
                np.broadcast_to(dinv_pos[cs], (P, S_PAD))),
            "peT": np.ascontiguousarray(peT_pos[:, cs]),
            "idxb": idx_buf,
            "tlocb": tloc_buf,
            "W1": W1b, "W2": W2, "b1": b1, "b2": b2,
        })

    geom = (TPC, HT0, sizes)
    return geom, in_maps, core_of, local_pos, n


def kernel(basic_block, edge_index, W1, b1, W2, b2):
    global _last_results
    geom, in_maps, core_of, local_pos, n = _prepare(
        basic_block, edge_index, W1, b1, W2, b2)

    if geom not in _PROGRAM_CACHE:
        _PROGRAM_CACHE[geom] = _build_program(geom)
    nc = _PROGRAM_CACHE[geom]

    res = None
    last_exc = None
    for attempt in range(3):
        try:
            res = bass_utils.run_bass_kernel_spmd(
                nc, in_maps, core_ids=list(range(N_CORES)))
            break
        except Exception as e:  # transient device/runtime hiccups
            last_exc = e
            import time
            time.sleep(5 * (attempt + 1))
    if res is None:
        raise last_exc
    _last_results = res

    out = np.empty((n, HID), dtype=np.float32)
    for c in range(N_CORES):
        nodes_c = np.nonzero(core_of == c)[0]
        out[nodes_c] = res.results[c]["outT"].T[local_pos[nodes_c]]
    return out

